# revision 5
# baseline (speedup 1.0000x reference)
"""Multi-head attention (B=2, L=2048, H=1024, NH=16) on 8 TRN2 NeuronCores.

Sharding: data-parallel over batch (2) x tensor-parallel over heads (4 groups
of 4 heads).  core = b*4 + g handles batch b, heads [4g, 4g+4).  Wq/Wk/Wv are
split column-wise, Wo row-wise; each core produces a partial [L, H] output
that the host sums per batch (the row-parallel all-reduce done host-side).

Device math (per core), all matmuls bf16 inputs / fp32 PSUM accumulation:
  QT = (Wq*0.125)^T x^T          [256, 2048]  (softmax scale folded into Wq)
  KT = Wk^T y^T                  [256, 2048]
  V  = y Wv                      [2048, 256] stored as V_aug [lk, 4*(64+1)]
                                 with a ones column per head
  per head h, per lq chunk:
    S^T[lk, lq] = KT_h^T QT_h    (contraction d=64; head pairs packed at
                                  partition offsets 0/64 -> PE row tiling)
    P^T = exp(S^T)               ScalarE, PSUM -> SBUF bf16
    O^T_aug[65, lq] = sum_lk V_aug_h^T P^T   (row 64 = softmax denominators)
    O'^T = O^T * broadcast(1/sums)           DVE recip + GpSimd partition bcast
  out[lq, 1024] += O'^T_cat^T Wo  (partial; host sums the 4 head-groups)
"""

import numpy as np
import ml_dtypes

B, L, H, NH, D = 2, 2048, 1024, 16, 64
GP = 4            # head-groups (tensor-parallel factor)
CH = H // GP      # 256 local projection cols per core
HL = NH // GP     # 4 local heads
LQ = 1024         # lq chunk size
NLQ = L // LQ
NKT = L // 128    # 16 lk tiles
BF16 = ml_dtypes.bfloat16

_CACHE = {}


def _build():
    import concourse.mybir as mybir
    import concourse.tile as tile
    from concourse import bacc

    dt = mybir.dt
    f32, bf16 = dt.float32, dt.bfloat16
    Exp = mybir.ActivationFunctionType.Exp

    nc = bacc.Bacc("TRN2", target_bir_lowering=False, debug=False)
    xT = nc.declare_dram_parameter("xT", [H, L], bf16, isOutput=False)
    yT = nc.declare_dram_parameter("yT", [H, L], bf16, isOutput=False)
    wq = nc.declare_dram_parameter("wq", [H, CH], bf16, isOutput=False)
    wk = nc.declare_dram_parameter("wk", [H, CH], bf16, isOutput=False)
    wv = nc.declare_dram_parameter("wv", [H, CH], bf16, isOutput=False)
    wo = nc.declare_dram_parameter("wo", [CH, H], bf16, isOutput=False)
    out = nc.declare_dram_parameter("out", [L, H], f32, isOutput=True)

    with tile.TileContext(nc) as tc:
        with (
            tc.tile_pool(name="w", bufs=1) as wpool,
            tc.tile_pool(name="acts", bufs=1) as apool,
            tc.tile_pool(name="psA", bufs=2, space="PSUM") as psA,
            tc.tile_pool(name="psO", bufs=2, space="PSUM") as psO,
            tc.tile_pool(name="pt", bufs=3) as ptpool,
            tc.tile_pool(name="oT", bufs=2) as otpool,
            tc.tile_pool(name="sm", bufs=3) as smpool,
            tc.tile_pool(name="osb", bufs=4) as opool,
        ):
            # ---- input DMAs (one dma_start per 128-row tile so each
            # consumer matmul waits on a single DMA-queue semaphore) -------
            wq_sb = wpool.tile([128, 8, CH], bf16, tag="wq")
            xT_sb = apool.tile([128, 8, L], bf16, tag="xT")
            wk_sb = wpool.tile([128, 8, CH], bf16, tag="wk")
            yT_sb = apool.tile([128, 8, L], bf16, tag="yT")
            wv_sb = wpool.tile([128, 8, CH], bf16, tag="wv")
            for sb, dram in ((wq_sb, wq), (xT_sb, xT), (wk_sb, wk),
                             (yT_sb, yT), (wv_sb, wv)):
                for t in range(8):
                    nc.sync.dma_start(
                        sb[:, t, :], dram[t * 128:(t + 1) * 128, :])
            wo_sb = wpool.tile([128, 2, H], bf16, tag="wo")
            for t in range(2):
                nc.sync.dma_start(
                    wo_sb[:, t, :], wo[t * 128:(t + 1) * 128, :])

            # ---- stage 1: projections ------------------------------------
            qT_sb = apool.tile([128, 2, L], bf16, tag="qT")
            kT_sb = apool.tile([128, 2, L], bf16, tag="kT")
            for w_sb, act_sb, dst in ((wq_sb, xT_sb, qT_sb), (wk_sb, yT_sb, kT_sb)):
                for ct in range(2):
                    for lh in range(L // LQ):
                        ps = psA.tile([128, LQ], f32, tag="psA")
                        for ht in range(8):
                            for sl in range(LQ // 512):
                                nc.tensor.matmul(
                                    ps[:, sl * 512:(sl + 1) * 512],
                                    lhsT=w_sb[:, ht, ct * 128:(ct + 1) * 128],
                                    rhs=act_sb[:, ht, lh * LQ + sl * 512:
                                               lh * LQ + (sl + 1) * 512],
                                    start=(ht == 0), stop=(ht == 7),
                                )
                        nc.vector.tensor_copy(
                            dst[:, ct, lh * LQ:(lh + 1) * LQ], ps)

            # V_aug: [lk, 4*(64+1)] bf16, ones column per head
            vaug_sb = apool.tile([128, NKT, HL * 65], bf16, tag="vaug")
            for lkt in range(NKT):
                psv = psA.tile([128, LQ], f32, tag="psA")
                for ht in range(8):
                    nc.tensor.matmul(
                        psv[:, :CH],
                        lhsT=yT_sb[:, ht, lkt * 128:(lkt + 1) * 128],
                        rhs=wv_sb[:, ht, :],
                        start=(ht == 0), stop=(ht == 7),
                    )
                vh = vaug_sb[:, lkt, :].rearrange("p (h e) -> p h e", h=HL)
                nc.vector.tensor_copy(
                    vh[:, :, 0:64],
                    psv[:, :CH].rearrange("p (h e) -> p h e", h=HL))
                nc.vector.memset(vh[:, :, 64], 1.0)

            # ---- stage 2 + 3 per lq chunk --------------------------------
            for ci in range(NLQ):
                oT_sb = otpool.tile([128, 2, LQ], bf16, tag="oT")
                for h in range(HL):
                    po = h % 2          # partition offset selector
                    ct2 = h // 2
                    psO_h = psO.tile([128, LQ], f32, tag="psO")
                    for lkt in range(NKT):
                        psS = psA.tile([128, LQ], f32, tag="psA")
                        for sl in range(LQ // 512):
                            nc.tensor.matmul(
                                psS[:, sl * 512:(sl + 1) * 512],
                                lhsT=kT_sb[64 * po:64 * po + 64, ct2,
                                           lkt * 128:(lkt + 1) * 128],
                                rhs=qT_sb[64 * po:64 * po + 64, ct2,
                                          ci * LQ + sl * 512:
                                          ci * LQ + (sl + 1) * 512],
                                start=True, stop=True,
                            )
                        pt = ptpool.tile([128, LQ], bf16, tag="pt")
                        nc.scalar.activation(pt, psS, Exp)
                        for sl in range(LQ // 512):
                            nc.tensor.matmul(
                                psO_h[0:65, sl * 512:(sl + 1) * 512],
                                lhsT=vaug_sb[:, lkt, h * 65:(h + 1) * 65],
                                rhs=pt[:, sl * 512:(sl + 1) * 512],
                                start=(lkt == 0), stop=(lkt == NKT - 1),
                            )
                    # normalize: O'^T = O^T / sums (sums = psO_h row 64)
                    recip = smpool.tile([1, LQ], f32, tag="recip")
                    nc.vector.reciprocal(recip, psO_h[64:65, :])
                    bcast = smpool.tile([64, LQ], f32, tag="bcast")
                    nc.gpsimd.partition_broadcast(bcast, recip)
                    nc.vector.tensor_mul(
                        oT_sb[64 * po:64 * po + 64, ct2, :],
                        psO_h[0:64, :], bcast)
                # stage 3: out[lq, :] = O'^T_cat^T Wo  (partial over heads)
                for mt in range(LQ // 128):
                    for nt in range(2):
                        pso = psO.tile([128, LQ], f32, tag="psO")
                        for kt in range(2):
                            nc.tensor.matmul(
                                pso[:, :512],
                                lhsT=oT_sb[:, kt, mt * 128:(mt + 1) * 128],
                                rhs=wo_sb[:, kt, nt * 512:(nt + 1) * 512],
                                start=(kt == 0), stop=(kt == 1),
                            )
                        osb = opool.tile([128, 512], f32, tag="osb")
                        nc.vector.tensor_copy(osb, pso[:, :512])
                        nc.sync.dma_start(
                            out[ci * LQ + mt * 128:ci * LQ + (mt + 1) * 128,
                                nt * 512:(nt + 1) * 512],
                            osb)
    nc.compile()
    return nc


def _get_nc():
    if "nc" not in _CACHE:
        _CACHE["nc"] = _build()
    return _CACHE["nc"]


def _in_maps(x, y, Wq, Wk, Wv, Wo):
    maps = []
    for core in range(8):
        b, g = core // GP, core % GP
        cs = slice(g * CH, (g + 1) * CH)
        maps.append({
            "xT": np.ascontiguousarray(x[b].T).astype(BF16),
            "yT": np.ascontiguousarray(y[b].T).astype(BF16),
            "wq": np.ascontiguousarray(Wq[:, cs] * np.float32(0.125)).astype(BF16),
            "wk": np.ascontiguousarray(Wk[:, cs]).astype(BF16),
            "wv": np.ascontiguousarray(Wv[:, cs]).astype(BF16),
            "wo": np.ascontiguousarray(Wo[cs, :]).astype(BF16),
        })
    return maps


def _install_ntff_hook():
    """Provide the antenv.axon_hooks shim missing from this container so
    run_bass_kernel_spmd(trace=True) can drive NTFF profiling via ctypes."""
    import sys
    import types
    try:
        from antenv.axon_hooks import get_axon_ntff_profile_hook  # noqa: F401
        return
    except ImportError:
        pass
    from trn_agent_boot.trn_boot import _ntff_profile_via_ctypes
    hook = _ntff_profile_via_ctypes("/opt/axon/libaxon_pjrt.so")
    mod = types.ModuleType("antenv.axon_hooks")
    mod.get_axon_ntff_profile_hook = lambda: hook
    mod.set_axon_ntff_profile_hook = lambda h: None
    sys.modules["antenv.axon_hooks"] = mod


def _run(inputs, trace=False):
    from concourse import bass_utils

    if trace:
        _install_ntff_hook()

    x, y, bias = inputs["x"], inputs["y"], inputs["bias"]
    if np.count_nonzero(np.asarray(bias)):
        raise NotImplementedError("nonzero attention bias not supported")
    nc = _get_nc()
    maps = _in_maps(np.asarray(x, np.float32), np.asarray(y, np.float32),
                    np.asarray(inputs["Wq"], np.float32),
                    np.asarray(inputs["Wk"], np.float32),
                    np.asarray(inputs["Wv"], np.float32),
                    np.asarray(inputs["Wo"], np.float32))
    res = bass_utils.run_bass_kernel_spmd(
        nc, maps, list(range(8)), trace=trace)
    out = np.zeros((B, L, H), np.float32)
    for core in range(8):
        out[core // GP] += res.results[core]["out"]
    return out, res


def kernel(**inputs):
    out, _ = _run(inputs, trace=False)
    return out


# revision 9
# speedup vs baseline: 1.0876x; 1.0876x over previous
"""Multi-head attention (B=2, L=2048, H=1024, NH=16) on 8 TRN2 NeuronCores.

Sharding: data-parallel over batch (2) x tensor-parallel over heads (4 groups
of 4 heads).  core = b*4 + g handles batch b, heads [4g, 4g+4).  Wq/Wk/Wv are
split column-wise, Wo row-wise; each core produces a partial [L, H] output
that the host sums per batch (the row-parallel all-reduce done host-side).

Device math (per core), all matmuls bf16 inputs / fp32 PSUM accumulation:
  QT = (Wq*0.125)^T x^T          [256, 2048]  (softmax scale folded into Wq)
  KT = Wk^T y^T                  [256, 2048]
  V  = y Wv                      [2048, 256] stored as V_aug [lk, 4*(64+1)]
                                 with a ones column per head
  per head h, per lq chunk:
    S^T[lk, lq] = KT_h^T QT_h    (contraction d=64; head pairs packed at
                                  partition offsets 0/64 -> PE row tiling)
    P^T = exp(S^T)               ScalarE, PSUM -> SBUF bf16
    O^T_aug[65, lq] = sum_lk V_aug_h^T P^T   (row 64 = softmax denominators)
    O'^T = O^T * broadcast(1/sums)           DVE recip + GpSimd partition bcast
  out[lq, 1024] += O'^T_cat^T Wo  (partial; host sums the 4 head-groups)
"""

import numpy as np
import ml_dtypes

B, L, H, NH, D = 2, 2048, 1024, 16, 64
GP = 4            # head-groups (tensor-parallel factor)
CH = H // GP      # 256 local projection cols per core
HL = NH // GP     # 4 local heads
LQ = 1024         # lq chunk size
NLQ = L // LQ
NKT = L // 128    # 16 lk tiles
BF16 = ml_dtypes.bfloat16

_CACHE = {}


def _build():
    import concourse.mybir as mybir
    import concourse.tile as tile
    from concourse import bacc

    dt = mybir.dt
    f32, bf16 = dt.float32, dt.bfloat16
    Exp = mybir.ActivationFunctionType.Exp

    nc = bacc.Bacc("TRN2", target_bir_lowering=False, debug=False)
    xT = nc.declare_dram_parameter("xT", [H, L], bf16, isOutput=False)
    yT = nc.declare_dram_parameter("yT", [H, L], bf16, isOutput=False)
    wq = nc.declare_dram_parameter("wq", [H, CH], bf16, isOutput=False)
    wk = nc.declare_dram_parameter("wk", [H, CH], bf16, isOutput=False)
    wv = nc.declare_dram_parameter("wv", [H, CH], bf16, isOutput=False)
    wo = nc.declare_dram_parameter("wo", [CH, H], bf16, isOutput=False)
    out = nc.declare_dram_parameter("out", [L, H], f32, isOutput=True)

    with tile.TileContext(nc) as tc:
        with (
            tc.tile_pool(name="w", bufs=1) as wpool,
            tc.tile_pool(name="acts", bufs=1) as apool,
            tc.tile_pool(name="psA", bufs=2, space="PSUM") as psA,
            tc.tile_pool(name="psO", bufs=2, space="PSUM") as psO,
            tc.tile_pool(name="pt", bufs=3) as ptpool,
            tc.tile_pool(name="oT", bufs=2) as otpool,
            tc.tile_pool(name="sm", bufs=3) as smpool,
            tc.tile_pool(name="osb", bufs=4) as opool,
        ):
            # ---- input DMAs (one dma_start per 128-row tile so each
            # consumer matmul waits on a single DMA-queue semaphore) -------
            wq_sb = wpool.tile([128, 8, CH], bf16, tag="wq")
            xT_sb = apool.tile([128, 8, L], bf16, tag="xT")
            wk_sb = wpool.tile([128, 8, CH], bf16, tag="wk")
            yT_sb = apool.tile([128, 8, L], bf16, tag="yT")
            wv_sb = wpool.tile([128, 8, CH], bf16, tag="wv")
            for sb, dram in ((wq_sb, wq), (wk_sb, wk), (wv_sb, wv)):
                for t in range(8):
                    nc.sync.dma_start(
                        sb[:, t, :], dram[t * 128:(t + 1) * 128, :])
            # activations split by l-half so the first projection psum
            # group can start after only half the tensor has landed
            for sb, dram in ((xT_sb, xT), (yT_sb, yT)):
                for lh in range(L // LQ):
                    for t in range(8):
                        nc.sync.dma_start(
                            sb[:, t, lh * LQ:(lh + 1) * LQ],
                            dram[t * 128:(t + 1) * 128, lh * LQ:(lh + 1) * LQ])
            wo_sb = wpool.tile([128, 2, H], bf16, tag="wo")
            for t in range(2):
                nc.sync.dma_start(
                    wo_sb[:, t, :], wo[t * 128:(t + 1) * 128, :])

            # ---- stage 1: projections ------------------------------------
            qT_sb = apool.tile([128, 2, L], bf16, tag="qT")
            kT_sb = apool.tile([128, 2, L], bf16, tag="kT")
            for w_sb, act_sb, dst in ((wq_sb, xT_sb, qT_sb), (wk_sb, yT_sb, kT_sb)):
                for ct in range(2):
                    for lh in range(L // LQ):
                        ps = psA.tile([128, LQ], f32, tag="psA")
                        for ht in range(8):
                            for sl in range(LQ // 512):
                                nc.tensor.matmul(
                                    ps[:, sl * 512:(sl + 1) * 512],
                                    lhsT=w_sb[:, ht, ct * 128:(ct + 1) * 128],
                                    rhs=act_sb[:, ht, lh * LQ + sl * 512:
                                               lh * LQ + (sl + 1) * 512],
                                    start=(ht == 0), stop=(ht == 7),
                                )
                        nc.vector.tensor_copy(
                            dst[:, ct, lh * LQ:(lh + 1) * LQ], ps)

            # V_aug: [lk, 4*(64+1)] bf16, ones column per head
            vaug_sb = apool.tile([128, NKT, HL * 65], bf16, tag="vaug")
            for lkt in range(NKT):
                psv = psA.tile([128, LQ], f32, tag="psA")
                for ht in range(8):
                    nc.tensor.matmul(
                        psv[:, :CH],
                        lhsT=yT_sb[:, ht, lkt * 128:(lkt + 1) * 128],
                        rhs=wv_sb[:, ht, :],
                        start=(ht == 0), stop=(ht == 7),
                    )
                vh = vaug_sb[:, lkt, :].rearrange("p (h e) -> p h e", h=HL)
                nc.vector.tensor_copy(
                    vh[:, :, 0:64],
                    psv[:, :CH].rearrange("p (h e) -> p h e", h=HL))
                nc.vector.memset(vh[:, :, 64], 1.0)

            # ---- stage 2 + 3 per lq chunk --------------------------------
            for ci in range(NLQ):
                oT_sb = otpool.tile([128, 2, LQ], bf16, tag="oT")
                for h in range(HL):
                    po = h % 2          # partition offset selector
                    ct2 = h // 2
                    psO_h = psO.tile([128, LQ], f32, tag="psO")
                    for lkt in range(NKT):
                        psS = psA.tile([128, LQ], f32, tag="psA")
                        for sl in range(LQ // 512):
                            nc.tensor.matmul(
                                psS[:, sl * 512:(sl + 1) * 512],
                                lhsT=kT_sb[64 * po:64 * po + 64, ct2,
                                           lkt * 128:(lkt + 1) * 128],
                                rhs=qT_sb[64 * po:64 * po + 64, ct2,
                                          ci * LQ + sl * 512:
                                          ci * LQ + (sl + 1) * 512],
                                start=True, stop=True,
                            )
                        pt = ptpool.tile([128, LQ], bf16, tag="pt")
                        nc.scalar.activation(pt, psS, Exp)
                        for sl in range(LQ // 512):
                            nc.tensor.matmul(
                                psO_h[0:65, sl * 512:(sl + 1) * 512],
                                lhsT=vaug_sb[:, lkt, h * 65:(h + 1) * 65],
                                rhs=pt[:, sl * 512:(sl + 1) * 512],
                                start=(lkt == 0), stop=(lkt == NKT - 1),
                            )
                    # normalize: O'^T = O^T / sums (sums = psO_h row 64).
                    # Copy PSUM->SBUF first so the psO slot frees after one
                    # DVE pass; the recip/bcast/mul chain then runs off the
                    # PSUM critical path.
                    ocp = smpool.tile([64, LQ], f32, tag="ocp")
                    nc.vector.tensor_copy(ocp, psO_h[0:64, :])
                    sums = smpool.tile([1, LQ], f32, tag="sums")
                    nc.vector.tensor_copy(sums, psO_h[64:65, :])
                    # NB: reciprocal_approx_fast misbehaves when its input AP
                    # has a non-zero base partition, hence the sums copy.
                    recip = smpool.tile([1, LQ], f32, tag="recip")
                    nc.vector.reciprocal_approx_fast(recip, sums)
                    bcast = smpool.tile([64, LQ], f32, tag="bcast")
                    nc.gpsimd.partition_broadcast(bcast, recip)
                    nc.vector.tensor_mul(
                        oT_sb[64 * po:64 * po + 64, ct2, :],
                        ocp, bcast)
                # stage 3: out[lq, :] = O'^T_cat^T Wo  (partial over heads)
                for mt in range(LQ // 128):
                    for nt in range(2):
                        pso = psO.tile([128, LQ], f32, tag="psO")
                        for kt in range(2):
                            nc.tensor.matmul(
                                pso[:, :512],
                                lhsT=oT_sb[:, kt, mt * 128:(mt + 1) * 128],
                                rhs=wo_sb[:, kt, nt * 512:(nt + 1) * 512],
                                start=(kt == 0), stop=(kt == 1),
                            )
                        osb = opool.tile([128, 512], f32, tag="osb")
                        nc.vector.tensor_copy(osb, pso[:, :512])
                        nc.sync.dma_start(
                            out[ci * LQ + mt * 128:ci * LQ + (mt + 1) * 128,
                                nt * 512:(nt + 1) * 512],
                            osb)
    nc.compile()
    return nc


def _get_nc():
    if "nc" not in _CACHE:
        _CACHE["nc"] = _build()
    return _CACHE["nc"]


def _in_maps(x, y, Wq, Wk, Wv, Wo):
    maps = []
    for core in range(8):
        b, g = core // GP, core % GP
        cs = slice(g * CH, (g + 1) * CH)
        maps.append({
            "xT": np.ascontiguousarray(x[b].T).astype(BF16),
            "yT": np.ascontiguousarray(y[b].T).astype(BF16),
            "wq": np.ascontiguousarray(Wq[:, cs] * np.float32(0.125)).astype(BF16),
            "wk": np.ascontiguousarray(Wk[:, cs]).astype(BF16),
            "wv": np.ascontiguousarray(Wv[:, cs]).astype(BF16),
            "wo": np.ascontiguousarray(Wo[cs, :]).astype(BF16),
        })
    return maps


def _install_ntff_hook():
    """Provide the antenv.axon_hooks shim missing from this container so
    run_bass_kernel_spmd(trace=True) can drive NTFF profiling via ctypes."""
    import sys
    import types
    try:
        from antenv.axon_hooks import get_axon_ntff_profile_hook  # noqa: F401
        return
    except ImportError:
        pass
    from trn_agent_boot.trn_boot import _ntff_profile_via_ctypes
    hook = _ntff_profile_via_ctypes("/opt/axon/libaxon_pjrt.so")
    mod = types.ModuleType("antenv.axon_hooks")
    mod.get_axon_ntff_profile_hook = lambda: hook
    mod.set_axon_ntff_profile_hook = lambda h: None
    sys.modules["antenv.axon_hooks"] = mod


def _run(inputs, trace=False):
    from concourse import bass_utils

    if trace:
        _install_ntff_hook()

    x, y, bias = inputs["x"], inputs["y"], inputs["bias"]
    if np.count_nonzero(np.asarray(bias)):
        raise NotImplementedError("nonzero attention bias not supported")
    nc = _get_nc()
    maps = _in_maps(np.asarray(x, np.float32), np.asarray(y, np.float32),
                    np.asarray(inputs["Wq"], np.float32),
                    np.asarray(inputs["Wk"], np.float32),
                    np.asarray(inputs["Wv"], np.float32),
                    np.asarray(inputs["Wo"], np.float32))
    res = bass_utils.run_bass_kernel_spmd(
        nc, maps, list(range(8)), trace=trace)
    out = np.zeros((B, L, H), np.float32)
    for core in range(8):
        out[core // GP] += res.results[core]["out"]
    return out, res


def kernel(**inputs):
    out, _ = _run(inputs, trace=False)
    return out
